# revision 41
# baseline (speedup 1.0000x reference)
"""BaiChuan attention block (QKV proj + RoPE + causal attention + o_proj) on 8 NeuronCores.

Sharding: tensor-parallel over heads. Each core owns 4 of the 32 heads:
W_pack columns (q/k/v slices) are column-sharded, w_o is row-sharded, and the
8 partial o_proj outputs are summed on the host (cheap f32 reduce).

Restructured vs the DRAM-bounce baseline:
  - qkv stays RESIDENT in SBUF (no DRAM round trip): phase-1 writes q/k as
    [d, s] head tiles and v DIRECTLY TRANSPOSED as [s, d] chunk tiles by
    swapping matmul operands (lhsT = hT chunk, rhs = w1_v), so attention needs
    no dma_start_transpose and no reload.
  - batch-serial schedule: S1 qkv(b0) | S2 attn(b0)+o_proj(b0) | S3 qkv(b1)
    | S4 attn(b1)+o_proj(b1). o_proj chunk emitters are popped one-per-j as
    TensorE fillers inside the attention j-loops.
  - attention j-loop software-pipelines the score matmul one chunk ahead
    (emission order SC(j+1), PV(j), SUM(j), filler), so the ScalarE exp
    latency for chunk j hides under ~1.5us of other TensorE work.
  - causal diagonal chunks are N-TRIMMED: score/exp/PV/SUM only cover the
    valid q-range [128r, 512), and the mask multiply shrinks to a [128,128]
    triangle.
  - softmax scale is folded into the exp (ACT free affine), so q and k share
    one unscaled cos/sin table pair.
  - the reciprocal row -> 128-partition broadcast runs on idle GpSimdE
    (partition_broadcast) instead of a K=1 TensorE matmul.
  - RoPE rotate-half copies are SBUF->SBUF DMAs; rope multiplies run in-place
    on the resident q/k tiles, emission-staggered so the rot DMA completes
    before VectorE needs it.
"""

import os
from collections import deque
import numpy as np
import ml_dtypes

try:
    # bass_utils imports this unconditionally when tracing under axon; some
    # images lack it. A None-hook stub makes tracing degrade gracefully
    # instead of raising ImportError.
    import antenv.axon_hooks  # noqa: F401
except Exception:
    import sys as _sys
    import types as _types
    _m = _types.ModuleType("antenv.axon_hooks")
    _m._hook = None
    _m.set_axon_ntff_profile_hook = lambda h: setattr(_m, "_hook", h)
    _m.get_axon_ntff_profile_hook = lambda: _m._hook
    _sys.modules["antenv.axon_hooks"] = _m
    try:
        import antenv as _antenv
        _antenv.axon_hooks = _m
    except Exception:
        pass

import concourse.bass as bass
import concourse.tile as tile
import concourse.mybir as mybir
from concourse import bacc
from concourse.bass_utils import run_bass_kernel_spmd

F32 = mybir.dt.float32
BF16 = mybir.dt.bfloat16
AF = mybir.ActivationFunctionType
BF = ml_dtypes.bfloat16

B, S, H = 2, 2048, 4096
BS = B * S                      # 4096 tokens
D = 128                         # head dim
NCORES = 8
NH_LOC = 4                      # heads per core (32 / 8)
HK = H // 128                   # 32 contraction chunks
ST = 512                        # seq tile / q-group width
GP = S // ST                    # 4 q-groups per batch
ROPE_THETA = 10000.0
SCALE = D ** -0.5

LAST_RESULT = None              # BassKernelResults of the most recent run (for test.py)


def _build_program():
    nc = bacc.Bacc()

    # all weight/activation inputs are host-prepacked partition-major so every
    # DMA lands as few large contiguous per-partition segments (>=1KB)
    hT = nc.dram_tensor("hT", [128, 8, HK, ST], BF16, kind="ExternalInput")
    w1 = nc.dram_tensor("w1", [128, 8, HK, 128], BF16, kind="ExternalInput")  # q|k heads
    w1v = nc.dram_tensor("w1v", [128, HK, 512], BF16, kind="ExternalInput")   # v cols
    wo = nc.dram_tensor("wo", [128, NH_LOC, H], BF16, kind="ExternalInput")
    cs = nc.dram_tensor("cs", [128, S], BF16, kind="ExternalInput")
    sn = nc.dram_tensor("sn", [128, S], BF16, kind="ExternalInput")
    maskd = nc.dram_tensor("mask", [128, 128], BF16, kind="ExternalInput")
    out = nc.dram_tensor("out", [H, BS], BF16, kind="ExternalOutput")

    with tile.TileContext(nc) as tc:
        with (
            tc.tile_pool(name="cons", bufs=1) as cons,
            tc.tile_pool(name="htp", bufs=3) as htp,
            tc.tile_pool(name="w1p", bufs=2) as w1p,
            tc.tile_pool(name="qkp", bufs=8) as qkp,
            tc.tile_pool(name="vp", bufs=16) as vp,
            tc.tile_pool(name="rotp", bufs=1) as rotp,
            tc.tile_pool(name="probsp", bufs=3) as probsp,
            tc.tile_pool(name="stagep", bufs=8) as stagep,
            tc.tile_pool(name="obp", bufs=3) as obp,
            tc.tile_pool(name="miscp", bufs=1) as miscp,
            tc.tile_pool(name="wop", bufs=1) as wop,
            tc.tile_pool(name="ps_acc", bufs=2, space="PSUM") as ps_acc,
            tc.tile_pool(name="ps_sc", bufs=3, space="PSUM") as ps_scp,
            tc.tile_pool(name="ps_out", bufs=2, space="PSUM") as ps_outp,
            tc.tile_pool(name="ps_sum", bufs=1, space="PSUM") as ps_sump,
        ):
            # ---- constants (gpsimd SWDGE queue: off the hot HWDGE queues) ----
            cs_sb = cons.tile([128, S], BF16, tag="cs")
            nc.gpsimd.dma_start(cs_sb[:], cs[:])
            sn_sb = cons.tile([128, S], BF16, tag="sn")
            nc.gpsimd.dma_start(sn_sb[:], sn[:])
            mask_sb = cons.tile([128, 128], BF16, tag="mask")
            nc.gpsimd.dma_start(mask_sb[:], maskd[:])
            ones_col = cons.tile([128, 1], BF16, tag="ones_col")
            nc.vector.memset(ones_col[:], 1.0)
            # dummy exp: pull the ACT exp-table load (~2.7us) out of the
            # first attention group and into the idle S1 ScalarE stream
            warm = cons.tile([1, 8], F32, tag="warm")
            nc.vector.memset(warm[:], 0.0)
            nc.scalar.activation(warm[:], warm[:], AF.Exp, scale=1.0)
            # PE warm-up: a few matmuls on memset data keep the PE busy (and
            # the HAM clock-gate open) from +0.3us while the first weight
            # chunk and ht granule are still in flight. Results land in a
            # dead PSUM row that the first attention group later overwrites.
            scratch0 = obp.tile([128, ST], BF16, tag="ob", name="scratch0")
            nc.vector.memset(scratch0[:], 0.0)
            ps_dummy = ps_sump.tile([1, ST], F32, tag="sum", name="ps_dummy")
            for i in range(8):
                nc.tensor.matmul(ps_dummy[:], ones_col[:], scratch0[:],
                                 start=(i == 0), stop=(i == 7))

            # v-part of w_pack and o_proj weights ride the idle SWDGE queue,
            # ordered by first use (w1v at ~55us, wo in S2), so the scalar
            # HWDGE queue only carries the per-chunk w1 loads
            w1v_sb = w1p.tile([128, HK, 512], BF16, tag="w1v", bufs=1)
            nc.gpsimd.dma_start(w1v_sb[:], w1v[:])
            wo_sb = wop.tile([128, NH_LOC, H], BF16, tag="wo")
            nc.gpsimd.dma_start(wo_sb[:], wo[:])

            filler = deque()

            def emit_filler():
                if filler:
                    filler.popleft()()

            # ---------------- phase 1 (per batch): qkv projection ----------------
            def emit_phase1(b, qk_tiles, v_tiles, attn_g0):
                g0_stages = []
                for tl in range(4):
                    t = 4 * b + tl
                    fwd = (tl % 2 == 0)
                    halves = (0, 1) if fwd else (1, 0)
                    pre_w1c = {}
                    if tl == 0:
                        # first two w1 chunks ahead of the ht loads on the
                        # scalar queue: the very first matmul gates on w1c m0
                        for m in range(2):
                            w1c = w1p.tile([128, HK, 128], BF16, tag="w1c",
                                           name=f"w1c_{t}_{m}")
                            nc.scalar.dma_start(w1c[:], w1[:, m])
                            pre_w1c[m] = w1c
                    ht_tiles = {}
                    # 1MB sub-DMAs: smaller granules pay ~2.3us per-DMA
                    # trigger/receipt overhead that halves effective load
                    # bandwidth on the cold t0 path
                    gran = 8
                    for hv in halves:
                        htt = htp.tile([128, 16, ST], BF16, tag="ht",
                                       name=f"ht_{t}_{hv}")
                        # all ht on the sync queue, emitted in consumption
                        # order: both HWDGE rings share the same 16 SDMA
                        # engines, so splitting buys no bandwidth — it only
                        # delays the w1 chunk stream that owns scalar
                        qs = range(0, 16, gran) if fwd else range(16 - gran, -1, -gran)
                        for q0_ in qs:
                            nc.sync.dma_start(
                                htt[:, q0_:q0_ + gran],
                                hT[:, t, hv * 16 + q0_: hv * 16 + q0_ + gran])
                        ht_tiles[hv] = htt
                    ko_order = list(range(HK)) if fwd else list(range(HK - 1, -1, -1))

                    def get_w1c(m):
                        if m in pre_w1c:
                            return pre_w1c.pop(m)
                        w1c = w1p.tile([128, HK, 128], BF16, tag="w1c",
                                       name=f"w1c_{t}_{m}")
                        nc.scalar.dma_start(w1c[:], w1[:, m])
                        return w1c

                    def qk_chunk(m, copy_eng):
                        w1c = get_w1c(m)
                        ps = ps_acc.tile([128, ST], F32, tag="acc",
                                         name=f"ps_p1_{t}_{m}")
                        for i, ko in enumerate(ko_order):
                            nc.tensor.matmul(
                                ps[:], w1c[:, ko], ht_tiles[ko // 16][:, ko % 16],
                                start=(i == 0), stop=(i == HK - 1))
                        dst = qk_tiles[('q' if m < 4 else 'k', m % 4)]
                        copy_eng(dst[:, tl * ST:(tl + 1) * ST], ps[:])

                    def v_chunk(sc, copy_eng):
                        ps = ps_acc.tile([128, ST], F32, tag="acc",
                                         name=f"ps_v_{t}_{sc}")
                        for i, ko in enumerate(ko_order):
                            nc.tensor.matmul(
                                ps[:],
                                ht_tiles[ko // 16][:, ko % 16, sc * 128:(sc + 1) * 128],
                                w1v_sb[:, ko],
                                start=(i == 0), stop=(i == HK - 1))
                        copy_eng(v_tiles[4 * tl + sc][:], ps[:])

                    if tl == 0 and b == 0:
                        # first two chains overlap the cold ht load: chain 0
                        # walks every granule as FIRST reader (fine-grained
                        # DMA-completion waits, arrival-paced); chain 1 shares
                        # granule 0 then re-reads the tiles once resident —
                        # its re-reads carry coarse whole-tile waits (Tile
                        # coalesces second-reader DMA deps), so running it
                        # last turns that into one short wait instead of a
                        # mid-pair stall that re-throttles HAM
                        w1c0, w1c1 = get_w1c(0), get_w1c(1)
                        ps0 = ps_acc.tile([128, ST], F32, tag="acc",
                                          name="ps_p1_0_0")
                        ps1 = ps_acc.tile([128, ST], F32, tag="acc",
                                          name="ps_p1_0_1")
                        for ko in ko_order[0:4]:
                            nc.tensor.matmul(
                                ps0[:], w1c0[:, ko], ht_tiles[ko // 16][:, ko % 16],
                                start=(ko == ko_order[0]), stop=False)
                            nc.tensor.matmul(
                                ps1[:], w1c1[:, ko], ht_tiles[ko // 16][:, ko % 16],
                                start=(ko == ko_order[0]), stop=False)
                        for ko in ko_order[4:]:
                            nc.tensor.matmul(
                                ps0[:], w1c0[:, ko], ht_tiles[ko // 16][:, ko % 16],
                                start=False, stop=(ko == ko_order[-1]))
                        for ko in ko_order[4:]:
                            nc.tensor.matmul(
                                ps1[:], w1c1[:, ko], ht_tiles[ko // 16][:, ko % 16],
                                start=False, stop=(ko == ko_order[-1]))
                        nc.vector.tensor_copy(qk_tiles[('q', 0)][:, 0:ST], ps0[:])
                        nc.vector.tensor_copy(qk_tiles[('q', 1)][:, 0:ST], ps1[:])
                        for m in range(2, 8):
                            qk_chunk(m, nc.vector.tensor_copy)
                        for sc in range(4):
                            v_chunk(sc, nc.scalar.copy)
                    elif tl < 3:
                        for m in range(8):
                            qk_chunk(m, nc.vector.tensor_copy)
                        for sc in range(4):
                            v_chunk(sc, nc.scalar.copy)
                    else:
                        # last t: q copies on VectorE, k copies on ScalarE —
                        # halves the ACT backlog sitting ahead of the first
                        # attention exps while leaving DVE slack for rope
                        for m in range(8):
                            qk_chunk(m, nc.vector.tensor_copy if m < 4
                                     else nc.scalar.copy)
                            _rot_dma(b, m, qk_tiles)
                            if m >= 1:
                                _rope_dve(b, m - 1, qk_tiles)
                        for sc in range(4):
                            v_chunk(sc, nc.vector.tensor_copy)
                        _rope_dve(b, 7, qk_tiles)
                return g0_stages

            rot_tiles = {}

            def _rot_dma(b, m, qk_tiles):
                kind = 'q' if m < 4 else 'k'
                x = qk_tiles[(kind, m % 4)]
                rot = rotp.tile([128, S], BF16, tag="rot", name=f"rot_{b}_{m}")
                nc.gpsimd.dma_start(rot[0:64, :], x[64:128, :])
                nc.gpsimd.dma_start(rot[64:128, :], x[0:64, :])
                rot_tiles[(b, m)] = rot

            def _rope_dve(b, m, qk_tiles):
                kind = 'q' if m < 4 else 'k'
                x = qk_tiles[(kind, m % 4)]
                rot = rot_tiles.pop((b, m))
                nc.vector.tensor_mul(rot[:], rot[:], sn_sb[:])
                nc.vector.tensor_mul(x[:], x[:], cs_sb[:])
                nc.vector.tensor_add(x[:], x[:], rot[:])

            # ---------------- attention (per batch, group, head) ----------------
            def emit_attn(b, g, h, qk_tiles, v_tiles):
                nj = 4 * g + 4
                q0 = g * ST
                qt = qk_tiles[('q', h)]
                kt = qk_tiles[('k', h)]
                ps_out = ps_outp.tile([128, ST], F32, tag="out",
                                      name=f"ps_out_{b}_{g}_{h}")
                ps_sum = ps_sump.tile([1, ST], F32, tag="sum",
                                      name=f"ps_sum_{b}_{g}_{h}")
                sc_tiles = {}

                def emit_sc(j):
                    r = j - 4 * g
                    c0 = 128 * r if r > 0 else 0
                    ps_sc = ps_scp.tile([128, ST], F32, tag="sc",
                                        name=f"ps_sc_{b}_{g}_{h}_{j}")
                    nc.tensor.matmul(ps_sc[:, c0:], kt[:, j * 128:(j + 1) * 128],
                                     qt[:, q0 + c0:q0 + ST], start=True, stop=True)
                    sc_tiles[j] = (ps_sc, c0)

                emit_sc(0)
                if nj > 1:
                    emit_sc(1)
                for j in range(nj):
                    if j + 2 < nj:
                        emit_sc(j + 2)
                    ps_sc, c0 = sc_tiles.pop(j)
                    probs = probsp.tile([128, ST], BF16, tag="probs",
                                        name=f"probs_{b}_{g}_{h}_{j}")
                    nc.scalar.activation(probs[:, c0:], ps_sc[:, c0:], AF.Exp,
                                         scale=SCALE)
                    if j - 4 * g >= 0:
                        nc.vector.tensor_mul(probs[:, c0:c0 + 128],
                                             probs[:, c0:c0 + 128], mask_sb[:])
                    nc.tensor.matmul(ps_out[:, c0:],
                                     v_tiles[j][:, h * 128:(h + 1) * 128],
                                     probs[:, c0:],
                                     start=(j == 0), stop=(j == nj - 1))
                    nc.tensor.matmul(ps_sum[:, c0:], ones_col[:], probs[:, c0:],
                                     start=(j == 0), stop=(j == nj - 1))
                    emit_filler()

                rec32 = miscp.tile([1, ST], F32, tag="rec32",
                                   name=f"rec32_{b}_{g}_{h}")
                nc.vector.reciprocal_approx_fast(rec32[:], ps_sum[:])
                rec16 = miscp.tile([1, ST], BF16, tag="rec16",
                                   name=f"rec16_{b}_{g}_{h}")
                nc.vector.tensor_copy(rec16[:], rec32[:])
                rbc = miscp.tile([128, ST], BF16, tag="rbc", bufs=2,
                                 name=f"rbc_{b}_{g}_{h}")
                nc.gpsimd.partition_broadcast(rbc[:], rec16[:], channels=128)
                stage = stagep.tile([128, ST], BF16, tag="stage",
                                    name=f"stage_{b}_{g}_{h}")
                nc.vector.tensor_mul(stage[:], ps_out[:], rbc[:])
                return stage

            # ---------------- o_proj chunk emitters (TensorE fillers) ----------------
            def make_oproj(t, m, stages):
                def emit():
                    ps = ps_acc.tile([128, ST], F32, tag="acc", name=f"ps_o_{t}_{m}")
                    for ko in range(NH_LOC):
                        nc.tensor.matmul(ps[:], wo_sb[:, ko, m * 128:(m + 1) * 128],
                                         stages[ko][:],
                                         start=(ko == 0), stop=(ko == NH_LOC - 1))
                    ob = obp.tile([128, ST], BF16, tag="ob", name=f"ob_{t}_{m}")
                    # all on DVE: ScalarE must stay a pure exp stream, it is
                    # the binding engine during filler-dry attention spells
                    nc.vector.tensor_copy(ob[:], ps[:])
                    # output DMAs ride the sync queue (idle sequencer; DIRECT2D
                    # triggers on scalar would delay exp issue) — except the
                    # drain-heavy last group of each batch, where both HWDGE
                    # queues share the flush and no exps remain to disturb
                    eng = nc.scalar if (t % 4 == 3 and m % 2 == 1) else nc.sync
                    eng.dma_start(
                        out[m * 128:(m + 1) * 128, t * ST:(t + 1) * ST], ob[:])
                return emit

            # ---------------- schedule ----------------
            for b in range(B):
                qk_tiles = {}
                for kind in ('q', 'k'):
                    for h in range(NH_LOC):
                        qk_tiles[(kind, h)] = qkp.tile(
                            [128, S], BF16, tag="qk", name=f"{kind}{h}_b{b}")
                v_tiles = [vp.tile([128, ST], BF16, tag="v", name=f"v{i}_b{b}")
                           for i in range(16)]
                emit_phase1(b, qk_tiles, v_tiles, None)
                for g in range(GP):
                    stages = [emit_attn(b, g, h, qk_tiles, v_tiles)
                              for h in range(NH_LOC)]
                    t = 4 * b + g
                    for m in range(H // 128):
                        filler.append(make_oproj(t, m, stages))
            while filler:
                filler.popleft()()

    nc.finalize()
    return nc


def _prep_inputs(positions, hidden_states, w_pack, w_o):
    pos = np.asarray(positions).astype(np.float32)
    hid = np.asarray(hidden_states, dtype=np.float32)
    w_pack = np.asarray(w_pack, dtype=np.float32)
    w_o = np.asarray(w_o, dtype=np.float32)

    # hT packed [p, t, ko, s']: hT_p[p, t, ko, s'] = hidden.T[ko*128+p, t*512+s']
    hTf = hid.reshape(BS, H).T.astype(BF)                       # [H, BS]
    hTp = np.ascontiguousarray(
        hTf.reshape(HK, 128, 8, ST).transpose(1, 2, 0, 3))      # [128, 8, HK, ST]

    inv_freq = 1.0 / (ROPE_THETA ** (np.arange(0, D, 2, dtype=np.float32) / D))
    ang = pos[None, :] * inv_freq[:, None]              # [64, S]
    cos = np.cos(ang).astype(np.float32)
    sin = np.sin(ang).astype(np.float32)
    cs = np.ascontiguousarray(np.concatenate([cos, cos], 0)).astype(BF)   # [128, S]
    sn = np.ascontiguousarray(np.concatenate([-sin, sin], 0)).astype(BF)

    # [128, 128] lower-triangle-inclusive: mask[k, q] = 1 if q >= k
    mask = (np.arange(128)[None, :] >= np.arange(128)[:, None]).astype(BF)

    in_maps = []
    for c in range(NCORES):
        j0 = 512 * c
        w1qk = np.concatenate([w_pack[:, j0:j0 + 512],
                               w_pack[:, H + j0:H + j0 + 512]], axis=1).astype(BF)
        # [H, 1024] -> [p, m, ko, c]
        w1p_ = np.ascontiguousarray(
            w1qk.reshape(HK, 128, 8, 128).transpose(1, 2, 0, 3))
        w1v_ = np.ascontiguousarray(
            w_pack[:, 2 * H + j0:2 * H + j0 + 512].astype(BF)
            .reshape(HK, 128, 512).transpose(1, 0, 2))          # [128, HK, 512]
        wo_ = np.ascontiguousarray(
            w_o[j0:j0 + 512, :].astype(BF)
            .reshape(NH_LOC, 128, H).transpose(1, 0, 2))        # [128, 4, H]
        in_maps.append({
            "hT": hTp, "w1": w1p_, "w1v": w1v_, "wo": wo_,
            "cs": cs, "sn": sn, "mask": mask,
        })
    return in_maps


def kernel(positions, hidden_states, w_pack, w_o):
    global LAST_RESULT
    nc = _build_program()
    in_maps = _prep_inputs(positions, hidden_states, w_pack, w_o)
    res = run_bass_kernel_spmd(
        nc, in_maps, core_ids=list(range(NCORES)),
        trace=bool(os.environ.get("BASS_TRACE")))
    LAST_RESULT = res
    acc = np.zeros((H, BS), np.float32)
    for r in res.results:
        acc += r["out"].astype(np.float32)
    return np.ascontiguousarray(acc.T).reshape(B, S, H)


# revision 46
# speedup vs baseline: 1.0551x; 1.0551x over previous
"""BaiChuan attention block (QKV proj + RoPE + causal attention + o_proj) on 8 NeuronCores.

Sharding: tensor-parallel over heads. Each core owns 4 of the 32 heads:
W_pack columns (q/k/v slices) are column-sharded, w_o is row-sharded, and the
8 partial o_proj outputs are summed on the host (cheap f32 reduce).

Restructured vs the DRAM-bounce baseline:
  - qkv stays RESIDENT in SBUF (no DRAM round trip): phase-1 writes q/k as
    [d, s] head tiles and v DIRECTLY TRANSPOSED as [s, d] chunk tiles by
    swapping matmul operands (lhsT = hT chunk, rhs = w1_v), so attention needs
    no dma_start_transpose and no reload.
  - batch-serial schedule: S1 qkv(b0) | S2 attn(b0)+o_proj(b0) | S3 qkv(b1)
    | S4 attn(b1)+o_proj(b1). o_proj chunk emitters are popped one-per-j as
    TensorE fillers inside the attention j-loops.
  - attention j-loop software-pipelines the score matmul one chunk ahead
    (emission order SC(j+1), PV(j), SUM(j), filler), so the ScalarE exp
    latency for chunk j hides under ~1.5us of other TensorE work.
  - causal diagonal chunks are N-TRIMMED: score/exp/PV/SUM only cover the
    valid q-range [128r, 512), and the mask multiply shrinks to a [128,128]
    triangle.
  - softmax scale is folded into the exp (ACT free affine), so q and k share
    one unscaled cos/sin table pair.
  - the reciprocal row -> 128-partition broadcast runs on idle GpSimdE
    (partition_broadcast) instead of a K=1 TensorE matmul.
  - RoPE rotate-half copies are SBUF->SBUF DMAs; rope multiplies run in-place
    on the resident q/k tiles, emission-staggered so the rot DMA completes
    before VectorE needs it.
"""

import os
from collections import deque
import numpy as np
import ml_dtypes

try:
    # bass_utils imports this unconditionally when tracing under axon; some
    # images lack it. A None-hook stub makes tracing degrade gracefully
    # instead of raising ImportError.
    import antenv.axon_hooks  # noqa: F401
except Exception:
    import sys as _sys
    import types as _types
    _m = _types.ModuleType("antenv.axon_hooks")
    _m._hook = None
    _m.set_axon_ntff_profile_hook = lambda h: setattr(_m, "_hook", h)
    _m.get_axon_ntff_profile_hook = lambda: _m._hook
    _sys.modules["antenv.axon_hooks"] = _m
    try:
        import antenv as _antenv
        _antenv.axon_hooks = _m
    except Exception:
        pass

import concourse.bass as bass
import concourse.tile as tile
import concourse.mybir as mybir
from concourse import bacc
from concourse.bass_utils import run_bass_kernel_spmd

F32 = mybir.dt.float32
BF16 = mybir.dt.bfloat16
AF = mybir.ActivationFunctionType
BF = ml_dtypes.bfloat16

B, S, H = 2, 2048, 4096
BS = B * S                      # 4096 tokens
D = 128                         # head dim
NCORES = 8
NH_LOC = 4                      # heads per core (32 / 8)
HK = H // 128                   # 32 contraction chunks
ST = 512                        # seq tile / q-group width
GP = S // ST                    # 4 q-groups per batch
ROPE_THETA = 10000.0
SCALE = D ** -0.5

LAST_RESULT = None              # BassKernelResults of the most recent run (for test.py)


def _build_program():
    nc = bacc.Bacc()

    # all weight/activation inputs are host-prepacked partition-major so every
    # DMA lands as few large contiguous per-partition segments (>=1KB)
    hT = nc.dram_tensor("hT", [128, 8, HK, ST], BF16, kind="ExternalInput")
    w1 = nc.dram_tensor("w1", [128, 8, HK, 128], BF16, kind="ExternalInput")  # q|k heads
    w1v = nc.dram_tensor("w1v", [128, HK, 512], BF16, kind="ExternalInput")   # v cols
    wo = nc.dram_tensor("wo", [128, NH_LOC, H], BF16, kind="ExternalInput")
    cs = nc.dram_tensor("cs", [128, S], BF16, kind="ExternalInput")
    sn = nc.dram_tensor("sn", [128, S], BF16, kind="ExternalInput")
    maskd = nc.dram_tensor("mask", [128, 128], BF16, kind="ExternalInput")
    out = nc.dram_tensor("out", [H, BS], BF16, kind="ExternalOutput")

    with tile.TileContext(nc) as tc:
        with (
            tc.tile_pool(name="cons", bufs=1) as cons,
            tc.tile_pool(name="htp", bufs=3) as htp,
            tc.tile_pool(name="w1p", bufs=2) as w1p,
            tc.tile_pool(name="qkp", bufs=8) as qkp,
            tc.tile_pool(name="vp", bufs=16) as vp,
            tc.tile_pool(name="rotp", bufs=1) as rotp,
            tc.tile_pool(name="probsp", bufs=3) as probsp,
            tc.tile_pool(name="stagep", bufs=8) as stagep,
            tc.tile_pool(name="obp", bufs=4) as obp,
            tc.tile_pool(name="miscp", bufs=1) as miscp,
            tc.tile_pool(name="wop", bufs=1) as wop,
            tc.tile_pool(name="ps_acc", bufs=2, space="PSUM") as ps_acc,
            tc.tile_pool(name="ps_sc", bufs=3, space="PSUM") as ps_scp,
            tc.tile_pool(name="ps_out", bufs=2, space="PSUM") as ps_outp,
            tc.tile_pool(name="ps_sum", bufs=1, space="PSUM") as ps_sump,
        ):
            # ---- constants (gpsimd SWDGE queue: off the hot HWDGE queues) ----
            cs_sb = cons.tile([128, S], BF16, tag="cs")
            nc.gpsimd.dma_start(cs_sb[:], cs[:])
            sn_sb = cons.tile([128, S], BF16, tag="sn")
            nc.gpsimd.dma_start(sn_sb[:], sn[:])
            mask_sb = cons.tile([128, 128], BF16, tag="mask")
            nc.gpsimd.dma_start(mask_sb[:], maskd[:])
            ones_col = cons.tile([128, 1], BF16, tag="ones_col")
            nc.vector.memset(ones_col[:], 1.0)
            # softmax-sum stationary: 128 replicated ones-columns. M=128 keeps
            # FWL eligible (a 1-column stationary disables it and costs ~90ns
            # per matmul), and every output row holds the same column-sum, so
            # the denominator lands already broadcast across partitions.
            ones128 = cons.tile([128, 128], BF16, tag="ones128")
            nc.vector.memset(ones128[:], 1.0)
            # dummy exp: pull the ACT exp-table load (~2.7us) out of the
            # first attention group and into the idle S1 ScalarE stream
            warm = cons.tile([1, 8], F32, tag="warm")
            nc.vector.memset(warm[:], 0.0)
            nc.scalar.activation(warm[:], warm[:], AF.Exp, scale=1.0)
            # PE warm-up: a few matmuls on memset data keep the PE busy (and
            # the HAM clock-gate open) from +0.3us while the first weight
            # chunk and ht granule are still in flight. Results land in a
            # dead PSUM row that the first attention group later overwrites.
            scratch0 = obp.tile([128, ST], BF16, tag="ob", name="scratch0")
            nc.vector.memset(scratch0[:], 0.0)
            ps_dummy = ps_sump.tile([1, ST], F32, tag="sum", name="ps_dummy")
            for i in range(8):
                nc.tensor.matmul(ps_dummy[:], ones_col[:], scratch0[:],
                                 start=(i == 0), stop=(i == 7))

            # v-part of w_pack and o_proj weights ride the idle SWDGE queue,
            # ordered by first use (w1v at ~55us, wo in S2), so the scalar
            # HWDGE queue only carries the per-chunk w1 loads
            w1v_sb = w1p.tile([128, HK, 512], BF16, tag="w1v", bufs=1)
            nc.gpsimd.dma_start(w1v_sb[:], w1v[:])
            wo_sb = wop.tile([128, NH_LOC, H], BF16, tag="wo")
            nc.gpsimd.dma_start(wo_sb[:], wo[:])

            filler = deque()

            def emit_filler():
                if filler:
                    filler.popleft()()

            # ---------------- phase 1 (per batch): qkv projection ----------------
            def emit_phase1(b, qk_tiles, v_tiles, attn_g0):
                g0_stages = []
                for tl in range(4):
                    t = 4 * b + tl
                    fwd = (tl % 2 == 0)
                    halves = (0, 1) if fwd else (1, 0)
                    pre_w1c = {}
                    if tl == 0:
                        # first two w1 chunks ahead of the ht loads on the
                        # scalar queue: the very first matmul gates on w1c m0
                        for m in range(2):
                            w1c = w1p.tile([128, HK, 128], BF16, tag="w1c",
                                           name=f"w1c_{t}_{m}")
                            nc.scalar.dma_start(w1c[:], w1[:, m])
                            pre_w1c[m] = w1c
                    ht_tiles = {}
                    # t0 is DMA-paced (nothing else to overlap): 4-ko granules
                    # so the first chains' waits track arrivals closely
                    gran = 4 if (b == 0 and tl == 0) else 8
                    for hv in halves:
                        htt = htp.tile([128, 16, ST], BF16, tag="ht",
                                       name=f"ht_{t}_{hv}")
                        # all ht on the sync queue, emitted in consumption
                        # order: both HWDGE rings share the same 16 SDMA
                        # engines, so splitting buys no bandwidth — it only
                        # delays the w1 chunk stream that owns scalar
                        qs = range(0, 16, gran) if fwd else range(16 - gran, -1, -gran)
                        for q0_ in qs:
                            nc.sync.dma_start(
                                htt[:, q0_:q0_ + gran],
                                hT[:, t, hv * 16 + q0_: hv * 16 + q0_ + gran])
                        ht_tiles[hv] = htt
                    ko_order = list(range(HK)) if fwd else list(range(HK - 1, -1, -1))

                    def get_w1c(m):
                        if m in pre_w1c:
                            return pre_w1c.pop(m)
                        w1c = w1p.tile([128, HK, 128], BF16, tag="w1c",
                                       name=f"w1c_{t}_{m}")
                        nc.scalar.dma_start(w1c[:], w1[:, m])
                        return w1c

                    def qk_chunk(m, copy_eng):
                        w1c = get_w1c(m)
                        ps = ps_acc.tile([128, ST], F32, tag="acc",
                                         name=f"ps_p1_{t}_{m}")
                        for i, ko in enumerate(ko_order):
                            nc.tensor.matmul(
                                ps[:], w1c[:, ko], ht_tiles[ko // 16][:, ko % 16],
                                start=(i == 0), stop=(i == HK - 1))
                        dst = qk_tiles[('q' if m < 4 else 'k', m % 4)]
                        copy_eng(dst[:, tl * ST:(tl + 1) * ST], ps[:])

                    def v_chunk(sc, copy_eng):
                        ps = ps_acc.tile([128, ST], F32, tag="acc",
                                         name=f"ps_v_{t}_{sc}")
                        for i, ko in enumerate(ko_order):
                            nc.tensor.matmul(
                                ps[:],
                                ht_tiles[ko // 16][:, ko % 16, sc * 128:(sc + 1) * 128],
                                w1v_sb[:, ko],
                                start=(i == 0), stop=(i == HK - 1))
                        copy_eng(v_tiles[4 * tl + sc][:], ps[:])

                    if tl == 0 and b == 0:
                        # first two chains overlap the cold ht load: chain 0
                        # walks every granule as FIRST reader (fine-grained
                        # DMA-completion waits, arrival-paced); chain 1 shares
                        # granule 0 then re-reads the tiles once resident —
                        # its re-reads carry coarse whole-tile waits (Tile
                        # coalesces second-reader DMA deps), so running it
                        # last turns that into one short wait instead of a
                        # mid-pair stall that re-throttles HAM
                        w1c0, w1c1 = get_w1c(0), get_w1c(1)
                        ps0 = ps_acc.tile([128, ST], F32, tag="acc",
                                          name="ps_p1_0_0")
                        ps1 = ps_acc.tile([128, ST], F32, tag="acc",
                                          name="ps_p1_0_1")
                        for ko in ko_order[0:4]:
                            nc.tensor.matmul(
                                ps0[:], w1c0[:, ko], ht_tiles[ko // 16][:, ko % 16],
                                start=(ko == ko_order[0]), stop=False)
                            nc.tensor.matmul(
                                ps1[:], w1c1[:, ko], ht_tiles[ko // 16][:, ko % 16],
                                start=(ko == ko_order[0]), stop=False)
                        for ko in ko_order[4:]:
                            nc.tensor.matmul(
                                ps0[:], w1c0[:, ko], ht_tiles[ko // 16][:, ko % 16],
                                start=False, stop=(ko == ko_order[-1]))
                        for ko in ko_order[4:]:
                            nc.tensor.matmul(
                                ps1[:], w1c1[:, ko], ht_tiles[ko // 16][:, ko % 16],
                                start=False, stop=(ko == ko_order[-1]))
                        nc.vector.tensor_copy(qk_tiles[('q', 0)][:, 0:ST], ps0[:])
                        nc.vector.tensor_copy(qk_tiles[('q', 1)][:, 0:ST], ps1[:])
                        for m in range(2, 8):
                            qk_chunk(m, nc.vector.tensor_copy)
                        for sc in range(4):
                            v_chunk(sc, nc.scalar.copy)
                    elif tl < 3:
                        for m in range(8):
                            qk_chunk(m, nc.vector.tensor_copy)
                        for sc in range(4):
                            v_chunk(sc, nc.scalar.copy)
                    else:
                        # last t: copies on ScalarE so VectorE is free for the
                        # staggered rope ops (rot DMA lands one m-chunk early)
                        for m in range(8):
                            qk_chunk(m, nc.scalar.copy)
                            _rot_dma(b, m, qk_tiles)
                            if m >= 1:
                                _rope_dve(b, m - 1, qk_tiles)
                        for sc in range(4):
                            v_chunk(sc, nc.vector.tensor_copy)
                        _rope_dve(b, 7, qk_tiles)
                return g0_stages

            rot_tiles = {}

            def _rot_dma(b, m, qk_tiles):
                kind = 'q' if m < 4 else 'k'
                x = qk_tiles[(kind, m % 4)]
                rot = rotp.tile([128, S], BF16, tag="rot", name=f"rot_{b}_{m}")
                nc.gpsimd.dma_start(rot[0:64, :], x[64:128, :])
                nc.gpsimd.dma_start(rot[64:128, :], x[0:64, :])
                rot_tiles[(b, m)] = rot

            def _rope_dve(b, m, qk_tiles):
                kind = 'q' if m < 4 else 'k'
                x = qk_tiles[(kind, m % 4)]
                rot = rot_tiles.pop((b, m))
                nc.vector.tensor_mul(rot[:], rot[:], sn_sb[:])
                nc.vector.tensor_mul(x[:], x[:], cs_sb[:])
                nc.vector.tensor_add(x[:], x[:], rot[:])

            # ---------------- attention (per batch, group, head) ----------------
            def emit_attn(b, g, h, qk_tiles, v_tiles):
                nj = 4 * g + 4
                q0 = g * ST
                qt = qk_tiles[('q', h)]
                kt = qk_tiles[('k', h)]
                ps_out = ps_outp.tile([128, ST], F32, tag="out",
                                      name=f"ps_out_{b}_{g}_{h}")
                ps_sum = ps_sump.tile([128, ST], F32, tag="sum",
                                      name=f"ps_sum_{b}_{g}_{h}")
                sc_tiles = {}

                def emit_sc(j):
                    r = j - 4 * g
                    c0 = 128 * r if r > 0 else 0
                    ps_sc = ps_scp.tile([128, ST], F32, tag="sc",
                                        name=f"ps_sc_{b}_{g}_{h}_{j}")
                    nc.tensor.matmul(ps_sc[:, c0:], kt[:, j * 128:(j + 1) * 128],
                                     qt[:, q0 + c0:q0 + ST], start=True, stop=True)
                    sc_tiles[j] = (ps_sc, c0)

                emit_sc(0)
                if nj > 1:
                    emit_sc(1)
                for j in range(nj):
                    if j + 2 < nj:
                        emit_sc(j + 2)
                    ps_sc, c0 = sc_tiles.pop(j)
                    probs = probsp.tile([128, ST], BF16, tag="probs",
                                        name=f"probs_{b}_{g}_{h}_{j}")
                    nc.scalar.activation(probs[:, c0:], ps_sc[:, c0:], AF.Exp,
                                         scale=SCALE)
                    if j - 4 * g >= 0:
                        nc.vector.tensor_mul(probs[:, c0:c0 + 128],
                                             probs[:, c0:c0 + 128], mask_sb[:])
                    nc.tensor.matmul(ps_out[:, c0:],
                                     v_tiles[j][:, h * 128:(h + 1) * 128],
                                     probs[:, c0:],
                                     start=(j == 0), stop=(j == nj - 1))
                    nc.tensor.matmul(ps_sum[:, c0:], ones128[:], probs[:, c0:],
                                     start=(j == 0), stop=(j == nj - 1))
                    emit_filler()

                # every ps_sum row is the same denominator: one full-lane
                # reciprocal replaces the old row-recip + bf16 copy + GpSimd
                # partition_broadcast chain
                rec32 = miscp.tile([128, ST], F32, tag="rec32", bufs=2,
                                   name=f"rec32_{b}_{g}_{h}")
                nc.vector.reciprocal_approx_fast(rec32[:], ps_sum[:])
                stage = stagep.tile([128, ST], BF16, tag="stage",
                                    name=f"stage_{b}_{g}_{h}")
                nc.vector.tensor_mul(stage[:], ps_out[:], rec32[:])
                return stage

            # ---------------- o_proj chunk emitters (TensorE fillers) ----------------
            def make_oproj(t, m, stages):
                def emit():
                    ps = ps_acc.tile([128, ST], F32, tag="acc", name=f"ps_o_{t}_{m}")
                    for ko in range(NH_LOC):
                        nc.tensor.matmul(ps[:], wo_sb[:, ko, m * 128:(m + 1) * 128],
                                         stages[ko][:],
                                         start=(ko == 0), stop=(ko == NH_LOC - 1))
                    ob = obp.tile([128, ST], BF16, tag="ob", name=f"ob_{t}_{m}")
                    # all on DVE: ScalarE must stay a pure exp stream, it is
                    # the binding engine during filler-dry attention spells
                    nc.vector.tensor_copy(ob[:], ps[:])
                    # output DMAs ride the sync queue (idle sequencer; DIRECT2D
                    # triggers on scalar would delay exp issue) — except the
                    # drain-heavy last group of each batch, where both HWDGE
                    # queues share the flush and no exps remain to disturb
                    eng = nc.scalar if (t % 4 == 3 and m % 2 == 1) else nc.sync
                    eng.dma_start(
                        out[m * 128:(m + 1) * 128, t * ST:(t + 1) * ST], ob[:])
                return emit

            # ---------------- schedule ----------------
            for b in range(B):
                qk_tiles = {}
                for kind in ('q', 'k'):
                    for h in range(NH_LOC):
                        qk_tiles[(kind, h)] = qkp.tile(
                            [128, S], BF16, tag="qk", name=f"{kind}{h}_b{b}")
                v_tiles = [vp.tile([128, ST], BF16, tag="v", name=f"v{i}_b{b}")
                           for i in range(16)]
                emit_phase1(b, qk_tiles, v_tiles, None)
                for g in range(GP):
                    stages = [emit_attn(b, g, h, qk_tiles, v_tiles)
                              for h in range(NH_LOC)]
                    t = 4 * b + g
                    for m in range(H // 128):
                        filler.append(make_oproj(t, m, stages))
            while filler:
                filler.popleft()()

    nc.finalize()
    return nc


def _prep_inputs(positions, hidden_states, w_pack, w_o):
    pos = np.asarray(positions).astype(np.float32)
    hid = np.asarray(hidden_states, dtype=np.float32)
    w_pack = np.asarray(w_pack, dtype=np.float32)
    w_o = np.asarray(w_o, dtype=np.float32)

    # hT packed [p, t, ko, s']: hT_p[p, t, ko, s'] = hidden.T[ko*128+p, t*512+s']
    hTf = hid.reshape(BS, H).T.astype(BF)                       # [H, BS]
    hTp = np.ascontiguousarray(
        hTf.reshape(HK, 128, 8, ST).transpose(1, 2, 0, 3))      # [128, 8, HK, ST]

    inv_freq = 1.0 / (ROPE_THETA ** (np.arange(0, D, 2, dtype=np.float32) / D))
    ang = pos[None, :] * inv_freq[:, None]              # [64, S]
    cos = np.cos(ang).astype(np.float32)
    sin = np.sin(ang).astype(np.float32)
    cs = np.ascontiguousarray(np.concatenate([cos, cos], 0)).astype(BF)   # [128, S]
    sn = np.ascontiguousarray(np.concatenate([-sin, sin], 0)).astype(BF)

    # [128, 128] lower-triangle-inclusive: mask[k, q] = 1 if q >= k
    mask = (np.arange(128)[None, :] >= np.arange(128)[:, None]).astype(BF)

    in_maps = []
    for c in range(NCORES):
        j0 = 512 * c
        w1qk = np.concatenate([w_pack[:, j0:j0 + 512],
                               w_pack[:, H + j0:H + j0 + 512]], axis=1).astype(BF)
        # [H, 1024] -> [p, m, ko, c]
        w1p_ = np.ascontiguousarray(
            w1qk.reshape(HK, 128, 8, 128).transpose(1, 2, 0, 3))
        w1v_ = np.ascontiguousarray(
            w_pack[:, 2 * H + j0:2 * H + j0 + 512].astype(BF)
            .reshape(HK, 128, 512).transpose(1, 0, 2))          # [128, HK, 512]
        wo_ = np.ascontiguousarray(
            w_o[j0:j0 + 512, :].astype(BF)
            .reshape(NH_LOC, 128, H).transpose(1, 0, 2))        # [128, 4, H]
        in_maps.append({
            "hT": hTp, "w1": w1p_, "w1v": w1v_, "wo": wo_,
            "cs": cs, "sn": sn, "mask": mask,
        })
    return in_maps


def kernel(positions, hidden_states, w_pack, w_o):
    global LAST_RESULT
    nc = _build_program()
    in_maps = _prep_inputs(positions, hidden_states, w_pack, w_o)
    res = run_bass_kernel_spmd(
        nc, in_maps, core_ids=list(range(NCORES)),
        trace=bool(os.environ.get("BASS_TRACE")))
    LAST_RESULT = res
    acc = np.zeros((H, BS), np.float32)
    for r in res.results:
        acc += r["out"].astype(np.float32)
    return np.ascontiguousarray(acc.T).reshape(B, S, H)
